# revision 29
# baseline (speedup 1.0000x reference)
"""Trainium2 Bass kernel for AdaptiveMixtureOfExperts (top-2 SwiGLU MoE).

Strategy (ff-sliced, fully balanced):
  - Host computes the tiny router (x @ Wr, top-2, softmax) with jax-on-CPU ops
    that bit-match the reference, then groups tokens by routed expert.
  - Every core processes EVERY expert's token set, but only a 1/8 slice of
    the FF dimension: core c owns ff neurons [c*256, (c+1)*256) of each
    expert (a-blocks 2c,2c+1 of W1 plus the matching g-blocks, and u-rows
    [c*256,(c+1)*256) of W2).  Section sizes are the true per-expert token
    counts -- identical on every core by construction, so the SPMD graph
    needs NO max-padding (the previous expert-pair layout streamed
    192*(maxbig+maxsmall)=405k columns/core; this streams 48*8192=393k,
    a ~5us saving at the bf16 roofline) and there is no pathological
    ragged tail (tiles are balanced to 342-512 columns).
        hT_slice = W1_sliceT @ xT        (4 psum blocks x 8 k-tiles)
        uT_slice = (a + b1a) * silu(g + b1g)          (2 k-tiles of u)
        y_partial = W2_sliceT @ uT_slice (8 m-blocks x 2 k-tiles)
  - Each core emits a bf16 PARTIAL y (contraction over its 256 u-rows) for
    all token-expert pairs, packed m-major per tile; the host sums the 8
    partials, applies b2 and the top-2 combine weights, and scatter-adds
    into the full [B, S, D] output.

DMA design notes (measured):
  - dma_start issue occupies its HWDGE sequencer ~600ns (128 descriptors)
    regardless of width, so one ring caps at ~206GB/s of 128KB transfers.
  - trn2 has exactly two fast HWDGE rings: qSP (sync) and qACT (scalar).
    The first section's supply is k-split across both (k0-3 on qSP, k4-7 on
    qACT); each ring pays its own ~3us cold ramp and the PE stream-follows
    from ~10.5us.  qACT is clear of issues before the first epilogue ACT.
  - gpsimd/vector dma_start is SWDGE (software Q7 descriptor gen): both
    uses tried in the previous layout REGRESSED (slow Q7 issue + HBM
    contention against the critical qSP head stream).
  - DMA-completion semaphores are 8 round-robin lanes shared by both
    rings; the 9th+ in-flight dma_start blocks on a lane recycling
    (transfer + ~1.2us HBM write receipt).  Fewer, wider early transfers
    beat many narrow ones -- xg moves as one full-section transfer per k.
  - Packed y output (one [128, 8*Nt] DMA per tile, m-major) rides qACT,
    which is otherwise ~50% idle; the global last tile uses per-m DMAs so
    the end-of-kernel exposure is one ~118KB transfer + receipt.
  - The chip clock-states between runs: N=512 matmul spacing is 216ns
    (2.4GHz) on some runs and 259ns (~2.0GHz P0 power state) on others --
    an ~18% exec-time lottery this kernel cannot control.  Compare runs
    only after checking the steady-state matmul delta.

Shapes hardcoded for the problem instance:
  x:[2,2048,1024] f32, Wr:[1024,8], temp:[1], W1:[8,1024,4096], b1:[8,4096],
  W2:[8,2048,1024], b2:[8,1024].  TOP_K=2, 8 experts on 8 cores.
"""

import os

import numpy as np
import ml_dtypes

D_MODEL = 1024
D_FF = 2048
NUM_EXPERTS = 8
TOP_K = 2
P = 128          # partitions
NT = 512         # max token tile (psum bank limit)
N_CORES = 8
FSL = D_FF // N_CORES      # 256 ff neurons per core slice
NO = D_MODEL // P          # 8 output row blocks
K1 = D_MODEL // P          # 8 k-tiles for matmul1
K2 = FSL // P              # 2 k-tiles for matmul2
WSL = 4 * P                # 512 w1 cols per (section, core): a|g|a|g

_NC_CACHE = {}
LAST_RESULTS = None  # test harness introspection


def _balanced_tiles(C):
    """Split C columns into ceil(C/NT) near-equal tiles (all <= NT)."""
    n_t = (C + NT - 1) // NT
    base, rem = divmod(C, n_t)
    sizes = [base + 1] * rem + [base] * (n_t - rem)
    bounds = []
    off = 0
    for sz in sizes:
        bounds.append((off, sz))
        off += sz
    return bounds


def _pad(n):
    # pad only to 4 columns (8-byte bf16 DMA lines); coarser padding streams
    # pure-zero columns through every matmul at ~80ns/column
    return max(P, ((n + 3) // 4) * 4)


def _build_nc(Cs):
    """Per-core Bass graph: NUM_EXPERTS ff-slice FFN sections of Cs[j] tokens.

    Identical on every core; only the dram inputs differ per core.
    """
    import concourse.mybir as mybir
    import concourse.tile as tile
    from concourse import bacc

    f32 = mybir.dt.float32
    bf16 = mybir.dt.bfloat16
    AF = mybir.ActivationFunctionType

    NS = len(Cs)
    Cmax = max(Cs)
    sec_bounds = [_balanced_tiles(C) for C in Cs]
    out_off = []
    w = 0
    for C in Cs:
        out_off.append(w)
        w += NO * C
    W_OUT = w

    nc = bacc.Bacc()
    xT = [
        nc.declare_dram_parameter(f"xT{j}", [D_MODEL, Cs[j]], bf16,
                                  isOutput=False)
        for j in range(NS)
    ]
    w1p = nc.declare_dram_parameter("w1p", [D_MODEL, NS * WSL], bf16,
                                    isOutput=False)
    w2p = nc.declare_dram_parameter("w2p", [NS * FSL, D_MODEL], bf16,
                                    isOutput=False)
    b1p = nc.declare_dram_parameter("b1p", [P, 4 * NS], f32, isOutput=False)
    outp = nc.declare_dram_parameter("outp", [P, W_OUT], bf16, isOutput=True)

    with tile.TileContext(nc) as tc:
        with (
            tc.tile_pool(name="weights", bufs=1) as wpool,
            tc.tile_pool(name="xgs", bufs=3) as xpool,
            tc.tile_pool(name="acts", bufs=2) as upool,
            tc.tile_pool(name="epilogue", bufs=4) as epool,
            tc.tile_pool(name="ps", bufs=8, space="PSUM") as ps_pool,
        ):
            b1_sb = wpool.tile([P, 4 * NS], f32, name="b1_sb", tag="b1")
            w1_sb = [
                wpool.tile([P, NS * WSL], bf16, name=f"w1_sb{k}", tag=f"w1{k}")
                for k in range(K1)
            ]
            w2_sb = [
                wpool.tile([P, D_MODEL], bf16, name=f"w2_sb{s2}",
                           tag=f"w2{s2}")
                for s2 in range(NS * K2)
            ]
            # xg tiles: uniform Cmax width so each k-tag rotates through 3
            # buffers (section j+3's DMA waits for mm1-j to release a slot;
            # with ~20us of compute per section the supply stays 2-3 sections
            # ahead of the PE).  Only Cs[j] columns are ever transferred.
            xg_sb = {}

            def xg_tile(j, k):
                t = xpool.tile([P, Cmax], bf16, name=f"xg{j}_{k}",
                               tag=f"xg{k}", bufs=2)
                xg_sb[(j, k)] = t
                return t

            # ---- input DMAs ----
            def emit_xg(j, eng):
                for k in range(K1):
                    eng.dma_start(
                        out=xg_tile(j, k)[:, 0:Cs[j]],
                        in_=xT[j][k * P:(k + 1) * P, 0:Cs[j]],
                    )

            def emit_w1(j, eng, ks=range(K1)):
                for k in ks:
                    eng.dma_start(
                        out=w1_sb[k][:, j * WSL:(j + 1) * WSL],
                        in_=w1p[k * P:(k + 1) * P, j * WSL:(j + 1) * WSL],
                    )

            def emit_w2(j, eng):
                for kk in range(K2):
                    s2 = j * K2 + kk
                    eng.dma_start(
                        out=w2_sb[s2][:],
                        in_=w2p[s2 * P:(s2 + 1) * P, :],
                    )

            # head: section 0 k-split across both HWDGE rings, xg+w1
            # interleaved per k in PE consumption order; b1p rides qACT
            # early (tiny, needed at the first ACT ~14us).
            Nt00 = sec_bounds[0][0][1]
            for k in range(K1):
                eng = nc.sync if k < K1 // 2 else nc.scalar
                # only tile 0's xg slice rides the critical head window;
                # the rest of section 0 follows right behind.
                eng.dma_start(
                    out=xg_tile(0, k)[:, 0:Nt00],
                    in_=xT[0][k * P:(k + 1) * P, 0:Nt00],
                )
                eng.dma_start(
                    out=w1_sb[k][:, 0:WSL],
                    in_=w1p[k * P:(k + 1) * P, 0:WSL],
                )
                if k == 5:
                    nc.scalar.dma_start(out=b1_sb[:], in_=b1t_ap(b1p))
            for k in range(K1):
                eng = nc.sync if k < K1 // 2 else nc.scalar
                eng.dma_start(
                    out=xg_sb[(0, k)][:, Nt00:Cs[0]],
                    in_=xT[0][k * P:(k + 1) * P, Nt00:Cs[0]],
                )
            # everything else on qSP in demand order
            emit_w2(0, nc.sync)
            for j in range(1, NS):
                emit_w1(j, nc.sync)
                emit_xg(j, nc.sync)
                emit_w2(j, nc.sync)

            # ---- compute ----
            def mm1(j):
                for t, (off, Nt) in enumerate(sec_bounds[j]):
                    uT = upool.tile([P, K2, NT], bf16, name=f"uT{j}_{t}",
                                    tag="uT", bufs=4)
                    uT_store[(j, t)] = uT
                    for p in range(K2):
                        base = j * WSL + 2 * P * p
                        ps_a = ps_pool.tile([P, NT], f32,
                                            name=f"psa{j}_{t}_{p}", tag="ps")
                        ps_g = ps_pool.tile([P, NT], f32,
                                            name=f"psg{j}_{t}_{p}", tag="ps")
                        for k in range(K1):
                            nc.tensor.matmul(
                                ps_a[:, :Nt],
                                w1_sb[k][:, base:base + P],
                                xg_sb[(j, k)][:, off:off + Nt],
                                start=(k == 0), stop=(k == K1 - 1),
                            )
                        for k in range(K1):
                            nc.tensor.matmul(
                                ps_g[:, :Nt],
                                w1_sb[k][:, base + P:base + 2 * P],
                                xg_sb[(j, k)][:, off:off + Nt],
                                start=(k == 0), stop=(k == K1 - 1),
                            )
                        bb = 4 * j + 2 * p
                        a_t = epool.tile([P, NT], bf16, name=f"a{j}_{t}_{p}",
                                         tag="a")
                        # a-side drain on DVE so ACT only carries silu-g:
                        # banks free ~2x faster when the drain backlog is
                        # what bounds the PE.
                        nc.vector.tensor_scalar_add(
                            a_t[:, :Nt], ps_a[:, :Nt], b1_sb[:, bb:bb + 1],
                        )
                        g_t = epool.tile([P, NT], bf16, name=f"g{j}_{t}_{p}",
                                         tag="g")
                        nc.scalar.activation(
                            g_t[:, :Nt], ps_g[:, :Nt], AF.Silu,
                            bias=b1_sb[:, bb + 1:bb + 2],
                        )
                        nc.vector.tensor_mul(
                            uT[:, p, :Nt], a_t[:, :Nt], g_t[:, :Nt])

            def mm2(j, last=False):
                for t, (off, Nt) in enumerate(sec_bounds[j]):
                    uT = uT_store.pop((j, t))
                    # the whole LAST section takes the qSP pair-packed path:
                    # any packed slab on the SWDGE queue near the kernel end
                    # gated the final barrier on SWDGE receipt semaphores
                    # (~2.5us observed), and a per-m path serialized 8 issues
                    is_last = last
                    o0 = out_off[j] + NO * off
                    y_w = None
                    if not is_last:
                        # 2D tile, m-blocks packed contiguously at Nt (not
                        # NT) stride: the output DMA src stays a contiguous
                        # 2D AP (a strided 3D DMA AP costs 3-8us of
                        # descriptor generation).
                        # bufs=3: with 2, tile t's drains wait on tile t-2's
                        # 1MB output DMA receipt (~3-4us); DVE/ACT are
                        # strict-FIFO so the whole drain chain and then the
                        # PE (PSUM banks) stall behind that wait.
                        y_w = epool.tile([P, NO * NT], bf16,
                                         name=f"yw{j}_{t}", tag="yw", bufs=4)
                    for m in range(NO):
                        ps_y = ps_pool.tile([P, NT], f32,
                                            name=f"psy{j}_{t}_{m}", tag="ps")
                        for kk in range(K2):
                            nc.tensor.matmul(
                                ps_y[:, :Nt],
                                w2_sb[j * K2 + kk][:, m * P:(m + 1) * P],
                                uT[:, kk, :Nt],
                                start=(kk == 0), stop=(kk == K2 - 1),
                            )
                        if is_last:
                            # last section: m-pairs packed contiguously at
                            # Nt stride into [P, 2*Nt] tiles, one qSP DMA
                            # per pair (236KB) -- 4 issues/tile, end-of-
                            # kernel exposure is one drain + issue +
                            # transfer + receipt.  Drains alternate DVE/ACT
                            # (different PSUM banks, parallel engines).
                            if m % 2 == 0:
                                y_p = epool.tile([P, 2 * NT], bf16,
                                                 name=f"yp{j}_{t}_{m}",
                                                 tag="yp", bufs=4)
                                nc.vector.tensor_copy(y_p[:, 0:Nt],
                                                      ps_y[:, :Nt])
                            else:
                                nc.scalar.activation(y_p[:, Nt:2 * Nt],
                                                     ps_y[:, :Nt],
                                                     AF.Identity)
                                nc.sync.dma_start(
                                    out=outp[:, o0 + (m - 1) * Nt:
                                             o0 + (m + 1) * Nt],
                                    in_=y_p[:, 0:2 * Nt],
                                )
                        elif m % 2 == 0:
                            # drains alternate DVE / ACT (different PSUM
                            # banks, parallel engines): mm2's drain demand
                            # (8 copies/tile) would otherwise outpace DVE.
                            nc.vector.tensor_copy(
                                y_w[:, m * Nt:(m + 1) * Nt], ps_y[:, :Nt])
                        else:
                            nc.scalar.activation(
                                y_w[:, m * Nt:(m + 1) * Nt], ps_y[:, :Nt],
                                AF.Identity)
                    if not is_last:
                        # one packed m-major DMA per tile.  Ring choice is
                        # phase-dependent: while the input stream owns the 8
                        # HWDGE completion-semaphore lanes (~first 70us), a
                        # qACT issue blocks the strict-FIFO drains ~5us per
                        # section waiting on a lane recycle, so early
                        # sections ride the idle GpSimd SWDGE queue; but the
                        # SWDGE ring drains slowly (its 16MB backlog gated
                        # the final barrier by ~3us), so once inputs are
                        # done, late sections ride qACT (lanes now free).
                        # qSP never carries slabs (in-order ring: +40us of
                        # input stalls when tried).
                        # j>=5 on qACT, not 4: on cores with slower input
                        # streams the lanes are still recycling inputs at
                        # ~120us and a section-4 qACT slab stalled the
                        # drains 8us.  Sections 3-4 ride qSP: by ~90us all
                        # input ISSUES are done, so a slab there delays only
                        # later slabs (early qSP slabs ahead of inputs cost
                        # +40us).  Only sections 0-2 (8.9MB) stay on the
                        # slow (~80GB/s) SWDGE ring so its backlog drains by
                        # ~136us -- with 0-4 on it, the final barrier waited
                        # ~3us on SWDGE receipts after compute ended.
                        if j < 4:
                            eng = nc.gpsimd
                        elif j < 5:
                            eng = nc.sync
                        else:
                            eng = nc.scalar
                        eng.dma_start(
                            out=outp[:, o0:o0 + NO * Nt],
                            in_=y_w[:, 0:NO * Nt])

            uT_store = {}
            for j in range(NS):
                mm1(j)
                mm2(j, last=(j == NS - 1))

    nc.compile()
    return nc


def b1t_ap(b1p):
    return b1p[:]


def _route_tokens(xf, Wr, temp):
    """Bit-match the reference's router on CPU jax: logits, top-2, softmax."""
    import jax
    import jax.numpy as jnp

    cpu = jax.devices("cpu")[0]
    with jax.default_device(cpu):
        xj = jnp.asarray(xf)
        logits = (xj @ jnp.asarray(Wr)) / jnp.asarray(temp)
        topw, topi = jax.lax.top_k(logits, TOP_K)
        topw = jax.nn.softmax(topw, axis=-1)
    return np.asarray(topi), np.asarray(topw)


def kernel(**inputs) -> np.ndarray:
    global LAST_RESULTS
    from concourse.bass_utils import run_bass_kernel_spmd

    x = np.asarray(inputs["x"], dtype=np.float32)
    Wr = np.asarray(inputs["Wr"], dtype=np.float32)
    temp = np.asarray(inputs["temp"], dtype=np.float32)
    W1 = np.asarray(inputs["W1"], dtype=np.float32)
    b1 = np.asarray(inputs["b1"], dtype=np.float32)
    W2 = np.asarray(inputs["W2"], dtype=np.float32)
    b2 = np.asarray(inputs["b2"], dtype=np.float32)

    B, S, D = x.shape
    T = B * S
    xf = x.reshape(T, D)

    topi, topw = _route_tokens(xf, Wr, temp)

    # Per-expert token lists and combine weights, sections sorted by
    # descending count (last section smallest -> smallest final tile).
    tok_idx = []
    tok_w = []
    for e in range(NUM_EXPERTS):
        mask = topi == e                       # [T, K]
        sel = mask.any(axis=1)
        idx = np.nonzero(sel)[0]
        w = (topw * mask).sum(axis=1)[idx]
        tok_idx.append(idx)
        tok_w.append(w.astype(np.float32))

    counts = np.array([len(i) for i in tok_idx])
    order = list(np.argsort(-counts, kind="stable"))
    Cs = tuple(_pad(counts[e]) for e in order)
    NS = len(order)

    bf16 = ml_dtypes.bfloat16

    # shared (all cores identical) x sections
    xT_arrs = {}
    for j, e in enumerate(order):
        idx = tok_idx[e]
        xg = np.zeros((Cs[j], D), dtype=np.float32)
        xg[: len(idx)] = xf[idx]
        xT_arrs[f"xT{j}"] = np.ascontiguousarray(xg.T).astype(bf16)

    in_maps = []
    for c in range(N_CORES):
        # w1 slice cols per section: [a_{2c} | g_{2c} | a_{2c+1} | g_{2c+1}]
        blocks = (2 * c, 2 * c + 1)
        w1_cols = []
        for jb in blocks:
            w1_cols.append(np.arange(jb * P, (jb + 1) * P))            # a_jb
            w1_cols.append(np.arange(D_FF + jb * P, D_FF + (jb + 1) * P))
        w1_cols = np.concatenate(w1_cols)
        m = dict(xT_arrs)
        m["w1p"] = np.ascontiguousarray(
            np.concatenate([W1[e][:, w1_cols] for e in order], axis=1)
        ).astype(bf16)
        m["w2p"] = np.ascontiguousarray(
            np.concatenate(
                [W2[e][c * FSL:(c + 1) * FSL, :] for e in order], axis=0)
        ).astype(bf16)
        m["b1p"] = np.ascontiguousarray(
            np.concatenate(
                [b1[e][w1_cols].reshape(4, P).T for e in order], axis=1)
        ).astype(np.float32)
        in_maps.append(m)

    if Cs not in _NC_CACHE:
        _NC_CACHE[Cs] = _build_nc(Cs)
    nc = _NC_CACHE[Cs]

    trace = bool(os.environ.get("MOE_KERNEL_TRACE"))
    kwargs = {}
    if trace:
        kwargs = dict(trace=True, trace_cores=list(range(N_CORES)))
    res = run_bass_kernel_spmd(nc, in_maps, core_ids=list(range(N_CORES)),
                               **kwargs)
    LAST_RESULTS = res

    out_off = []
    w = 0
    for C in Cs:
        out_off.append(w)
        w += NO * C

    out = np.zeros((T, D), dtype=np.float32)
    for j, e in enumerate(order):
        idx = tok_idx[e]
        if len(idx) == 0:
            continue
        C = Cs[j]
        acc = np.zeros((P, NO * C), dtype=np.float32)
        for c in range(N_CORES):
            acc += np.asarray(
                res.results[c]["outp"][:, out_off[j]:out_off[j] + NO * C]
            ).astype(np.float32)
        # per-tile m-major slabs -> [D_MODEL, C]
        y = np.empty((D_MODEL, C), dtype=np.float32)
        for off, Nt in _balanced_tiles(C):
            slab = acc[:, NO * off:NO * (off + Nt)].reshape(P, NO, Nt)
            y[:, off:off + Nt] = slab.transpose(1, 0, 2).reshape(D_MODEL, Nt)
        yt = y[:, : len(idx)].T + b2[e]
        out[idx] += yt * tok_w[e][:, None]

    return out.reshape(B, S, D)


# revision 31
# speedup vs baseline: 1.1243x; 1.1243x over previous
"""Trainium2 Bass kernel for AdaptiveMixtureOfExperts (top-2 SwiGLU MoE).

Strategy (ff-sliced, fully balanced):
  - Host computes the tiny router (x @ Wr, top-2, softmax) with jax-on-CPU ops
    that bit-match the reference, then groups tokens by routed expert.
  - Every core processes EVERY expert's token set, but only a 1/8 slice of
    the FF dimension: core c owns ff neurons [c*256, (c+1)*256) of each
    expert (a-blocks 2c,2c+1 of W1 plus the matching g-blocks, and u-rows
    [c*256,(c+1)*256) of W2).  Section sizes are the true per-expert token
    counts -- identical on every core by construction, so the SPMD graph
    needs NO max-padding (the previous expert-pair layout streamed
    192*(maxbig+maxsmall)=405k columns/core; this streams 48*8192=393k,
    a ~5us saving at the bf16 roofline) and there is no pathological
    ragged tail (tiles are balanced to 342-512 columns).
        hT_slice = W1_sliceT @ xT        (4 psum blocks x 8 k-tiles)
        uT_slice = (a + b1a) * silu(g + b1g)          (2 k-tiles of u)
        y_partial = W2_sliceT @ uT_slice (8 m-blocks x 2 k-tiles)
  - Each core emits a bf16 PARTIAL y (contraction over its 256 u-rows) for
    all token-expert pairs, packed m-major per tile; the host sums the 8
    partials, applies b2 and the top-2 combine weights, and scatter-adds
    into the full [B, S, D] output.

DMA design notes (measured):
  - dma_start issue occupies its HWDGE sequencer ~600ns (128 descriptors)
    regardless of width, so one ring caps at ~206GB/s of 128KB transfers.
  - trn2 has exactly two fast HWDGE rings: qSP (sync) and qACT (scalar).
    The first section's supply is k-split across both (k0-3 on qSP, k4-7 on
    qACT); each ring pays its own ~3us cold ramp and the PE stream-follows
    from ~10.5us.  qACT is clear of issues before the first epilogue ACT.
  - gpsimd/vector dma_start is SWDGE (software Q7 descriptor gen): both
    uses tried in the previous layout REGRESSED (slow Q7 issue + HBM
    contention against the critical qSP head stream).
  - DMA-completion semaphores are 8 round-robin lanes shared by both
    rings; the 9th+ in-flight dma_start blocks on a lane recycling
    (transfer + ~1.2us HBM write receipt).  Fewer, wider early transfers
    beat many narrow ones -- xg moves as one full-section transfer per k.
  - Packed y output (one [128, 8*Nt] DMA per tile, m-major) rides qACT,
    which is otherwise ~50% idle; the global last tile uses per-m DMAs so
    the end-of-kernel exposure is one ~118KB transfer + receipt.
  - The chip clock-states between runs: N=512 matmul spacing is 216ns
    (2.4GHz) on some runs and 259ns (~2.0GHz P0 power state) on others --
    an ~18% exec-time lottery this kernel cannot control.  Compare runs
    only after checking the steady-state matmul delta.

Shapes hardcoded for the problem instance:
  x:[2,2048,1024] f32, Wr:[1024,8], temp:[1], W1:[8,1024,4096], b1:[8,4096],
  W2:[8,2048,1024], b2:[8,1024].  TOP_K=2, 8 experts on 8 cores.
"""

import os

import numpy as np
import ml_dtypes

D_MODEL = 1024
D_FF = 2048
NUM_EXPERTS = 8
TOP_K = 2
P = 128          # partitions
NT = 512         # max token tile (psum bank limit)
N_CORES = 8
FSL = D_FF // N_CORES      # 256 ff neurons per core slice
NO = D_MODEL // P          # 8 output row blocks
K1 = D_MODEL // P          # 8 k-tiles for matmul1
K2 = FSL // P              # 2 k-tiles for matmul2
WSL = 4 * P                # 512 w1 cols per (section, core): a|g|a|g

_NC_CACHE = {}
LAST_RESULTS = None  # test harness introspection


def _balanced_tiles(C):
    """Split C columns into ceil(C/NT) near-equal tiles (all <= NT)."""
    n_t = (C + NT - 1) // NT
    base, rem = divmod(C, n_t)
    sizes = [base + 1] * rem + [base] * (n_t - rem)
    bounds = []
    off = 0
    for sz in sizes:
        bounds.append((off, sz))
        off += sz
    return bounds


def _pad(n):
    # pad only to 4 columns (8-byte bf16 DMA lines); coarser padding streams
    # pure-zero columns through every matmul at ~80ns/column
    return max(P, ((n + 3) // 4) * 4)


def _build_nc(Cs):
    """Per-core Bass graph: NUM_EXPERTS ff-slice FFN sections of Cs[j] tokens.

    Identical on every core; only the dram inputs differ per core.
    """
    import concourse.mybir as mybir
    import concourse.tile as tile
    from concourse import bacc

    f32 = mybir.dt.float32
    bf16 = mybir.dt.bfloat16
    AF = mybir.ActivationFunctionType

    NS = len(Cs)
    Cmax = max(Cs)
    sec_bounds = [_balanced_tiles(C) for C in Cs]
    out_off = []
    w = 0
    for C in Cs:
        out_off.append(w)
        w += NO * C
    W_OUT = w

    nc = bacc.Bacc()
    xT = [
        nc.declare_dram_parameter(f"xT{j}", [D_MODEL, Cs[j]], bf16,
                                  isOutput=False)
        for j in range(NS)
    ]
    w1p = nc.declare_dram_parameter("w1p", [D_MODEL, NS * WSL], bf16,
                                    isOutput=False)
    w2p = nc.declare_dram_parameter("w2p", [NS * FSL, D_MODEL], bf16,
                                    isOutput=False)
    b1p = nc.declare_dram_parameter("b1p", [P, 4 * NS], f32, isOutput=False)
    outp = nc.declare_dram_parameter("outp", [P, W_OUT], bf16, isOutput=True)

    with tile.TileContext(nc) as tc:
        with (
            tc.tile_pool(name="weights", bufs=1) as wpool,
            tc.tile_pool(name="xgs", bufs=3) as xpool,
            tc.tile_pool(name="acts", bufs=2) as upool,
            tc.tile_pool(name="epilogue", bufs=4) as epool,
            tc.tile_pool(name="ps", bufs=8, space="PSUM") as ps_pool,
        ):
            b1_sb = wpool.tile([P, 4 * NS], f32, name="b1_sb", tag="b1")
            w1_sb = [
                wpool.tile([P, NS * WSL], bf16, name=f"w1_sb{k}", tag=f"w1{k}")
                for k in range(K1)
            ]
            w2_sb = [
                wpool.tile([P, D_MODEL], bf16, name=f"w2_sb{s2}",
                           tag=f"w2{s2}")
                for s2 in range(NS * K2)
            ]
            # xg tiles: uniform Cmax width so each k-tag rotates through 3
            # buffers (section j+3's DMA waits for mm1-j to release a slot;
            # with ~20us of compute per section the supply stays 2-3 sections
            # ahead of the PE).  Only Cs[j] columns are ever transferred.
            xg_sb = {}

            def xg_tile(j, k):
                t = xpool.tile([P, Cmax], bf16, name=f"xg{j}_{k}",
                               tag=f"xg{k}", bufs=2)
                xg_sb[(j, k)] = t
                return t

            # ---- input DMAs ----
            def emit_xg(j, eng):
                for k in range(K1):
                    eng.dma_start(
                        out=xg_tile(j, k)[:, 0:Cs[j]],
                        in_=xT[j][k * P:(k + 1) * P, 0:Cs[j]],
                    )

            def emit_w1(j, eng, ks=range(K1)):
                for k in ks:
                    eng.dma_start(
                        out=w1_sb[k][:, j * WSL:(j + 1) * WSL],
                        in_=w1p[k * P:(k + 1) * P, j * WSL:(j + 1) * WSL],
                    )

            def emit_w2(j, eng):
                for kk in range(K2):
                    s2 = j * K2 + kk
                    eng.dma_start(
                        out=w2_sb[s2][:],
                        in_=w2p[s2 * P:(s2 + 1) * P, :],
                    )

            # head: section 0 k-split across both HWDGE rings, xg+w1
            # interleaved per k in PE consumption order; b1p rides qACT
            # early (tiny, needed at the first ACT ~14us).
            Nt00 = sec_bounds[0][0][1]
            for k in range(K1):
                eng = nc.sync if k < K1 // 2 else nc.scalar
                # only tile 0's xg slice rides the critical head window;
                # the rest of section 0 follows right behind.
                eng.dma_start(
                    out=xg_tile(0, k)[:, 0:Nt00],
                    in_=xT[0][k * P:(k + 1) * P, 0:Nt00],
                )
                eng.dma_start(
                    out=w1_sb[k][:, 0:WSL],
                    in_=w1p[k * P:(k + 1) * P, 0:WSL],
                )
                if k == 5:
                    nc.scalar.dma_start(out=b1_sb[:], in_=b1t_ap(b1p))
            for k in range(K1):
                eng = nc.sync if k < K1 // 2 else nc.scalar
                eng.dma_start(
                    out=xg_sb[(0, k)][:, Nt00:Cs[0]],
                    in_=xT[0][k * P:(k + 1) * P, Nt00:Cs[0]],
                )
            # everything else on qSP in demand order
            emit_w2(0, nc.sync)
            for j in range(1, NS):
                emit_w1(j, nc.sync)
                emit_xg(j, nc.sync)
                emit_w2(j, nc.sync)

            # ---- compute ----
            def mm1(j):
                for t, (off, Nt) in enumerate(sec_bounds[j]):
                    uT = upool.tile([P, K2, NT], bf16, name=f"uT{j}_{t}",
                                    tag="uT", bufs=4)
                    uT_store[(j, t)] = uT
                    for p in range(K2):
                        base = j * WSL + 2 * P * p
                        ps_a = ps_pool.tile([P, NT], f32,
                                            name=f"psa{j}_{t}_{p}", tag="ps")
                        ps_g = ps_pool.tile([P, NT], f32,
                                            name=f"psg{j}_{t}_{p}", tag="ps")
                        for k in range(K1):
                            nc.tensor.matmul(
                                ps_a[:, :Nt],
                                w1_sb[k][:, base:base + P],
                                xg_sb[(j, k)][:, off:off + Nt],
                                start=(k == 0), stop=(k == K1 - 1),
                            )
                        for k in range(K1):
                            nc.tensor.matmul(
                                ps_g[:, :Nt],
                                w1_sb[k][:, base + P:base + 2 * P],
                                xg_sb[(j, k)][:, off:off + Nt],
                                start=(k == 0), stop=(k == K1 - 1),
                            )
                        bb = 4 * j + 2 * p
                        a_t = epool.tile([P, NT], bf16, name=f"a{j}_{t}_{p}",
                                         tag="a")
                        # a-side drain on DVE so ACT only carries silu-g:
                        # banks free ~2x faster when the drain backlog is
                        # what bounds the PE.
                        nc.vector.tensor_scalar_add(
                            a_t[:, :Nt], ps_a[:, :Nt], b1_sb[:, bb:bb + 1],
                        )
                        g_t = epool.tile([P, NT], bf16, name=f"g{j}_{t}_{p}",
                                         tag="g")
                        nc.scalar.activation(
                            g_t[:, :Nt], ps_g[:, :Nt], AF.Silu,
                            bias=b1_sb[:, bb + 1:bb + 2],
                        )
                        nc.vector.tensor_mul(
                            uT[:, p, :Nt], a_t[:, :Nt], g_t[:, :Nt])

            def mm2(j, last=False):
                for t, (off, Nt) in enumerate(sec_bounds[j]):
                    uT = uT_store.pop((j, t))
                    # the whole LAST section takes the qSP pair-packed path:
                    # any packed slab on the SWDGE queue near the kernel end
                    # gated the final barrier on SWDGE receipt semaphores
                    # (~2.5us observed), and a per-m path serialized 8 issues
                    is_last = last
                    o0 = out_off[j] + NO * off
                    y_w = None
                    if not is_last:
                        # 2D tile, m-blocks packed contiguously at Nt (not
                        # NT) stride: the output DMA src stays a contiguous
                        # 2D AP (a strided 3D DMA AP costs 3-8us of
                        # descriptor generation).
                        # bufs=3: with 2, tile t's drains wait on tile t-2's
                        # 1MB output DMA receipt (~3-4us); DVE/ACT are
                        # strict-FIFO so the whole drain chain and then the
                        # PE (PSUM banks) stall behind that wait.
                        y_w = epool.tile([P, NO * NT], bf16,
                                         name=f"yw{j}_{t}", tag="yw", bufs=4)
                    for m in range(NO):
                        ps_y = ps_pool.tile([P, NT], f32,
                                            name=f"psy{j}_{t}_{m}", tag="ps")
                        for kk in range(K2):
                            nc.tensor.matmul(
                                ps_y[:, :Nt],
                                w2_sb[j * K2 + kk][:, m * P:(m + 1) * P],
                                uT[:, kk, :Nt],
                                start=(kk == 0), stop=(kk == K2 - 1),
                            )
                        if is_last:
                            # last section: m-pairs packed contiguously at
                            # Nt stride into [P, 2*Nt] tiles, one qSP DMA
                            # per pair (236KB) -- 4 issues/tile, end-of-
                            # kernel exposure is one drain + issue +
                            # transfer + receipt.  Drains alternate DVE/ACT
                            # (different PSUM banks, parallel engines).
                            if m % 2 == 0:
                                y_p = epool.tile([P, 2 * NT], bf16,
                                                 name=f"yp{j}_{t}_{m}",
                                                 tag="yp", bufs=4)
                                nc.vector.tensor_copy(y_p[:, 0:Nt],
                                                      ps_y[:, :Nt])
                            else:
                                nc.scalar.activation(y_p[:, Nt:2 * Nt],
                                                     ps_y[:, :Nt],
                                                     AF.Identity)
                                nc.sync.dma_start(
                                    out=outp[:, o0 + (m - 1) * Nt:
                                             o0 + (m + 1) * Nt],
                                    in_=y_p[:, 0:2 * Nt],
                                )
                        elif m % 2 == 0:
                            # drains alternate DVE / ACT (different PSUM
                            # banks, parallel engines): mm2's drain demand
                            # (8 copies/tile) would otherwise outpace DVE.
                            nc.vector.tensor_copy(
                                y_w[:, m * Nt:(m + 1) * Nt], ps_y[:, :Nt])
                        else:
                            nc.scalar.activation(
                                y_w[:, m * Nt:(m + 1) * Nt], ps_y[:, :Nt],
                                AF.Identity)
                    if not is_last:
                        # one packed m-major DMA per tile.  Ring choice is
                        # phase-dependent: while the input stream owns the 8
                        # HWDGE completion-semaphore lanes (~first 70us), a
                        # qACT issue blocks the strict-FIFO drains ~5us per
                        # section waiting on a lane recycle, so early
                        # sections ride the idle GpSimd SWDGE queue; but the
                        # SWDGE ring drains slowly (its 16MB backlog gated
                        # the final barrier by ~3us), so once inputs are
                        # done, late sections ride qACT (lanes now free).
                        # qSP never carries slabs (in-order ring: +40us of
                        # input stalls when tried).
                        # j>=5 on qACT, not 4: on cores with slower input
                        # streams the lanes are still recycling inputs at
                        # ~120us and a section-4 qACT slab stalled the
                        # drains 8us.  Sections 3-4 ride qSP: by ~90us all
                        # input ISSUES are done, so a slab there delays only
                        # later slabs (early qSP slabs ahead of inputs cost
                        # +40us).  Only sections 0-2 (8.9MB) stay on the
                        # slow (~80GB/s) SWDGE ring so its backlog drains by
                        # ~136us -- with 0-4 on it, the final barrier waited
                        # ~3us on SWDGE receipts after compute ended.
                        if j < 4:
                            eng = nc.gpsimd
                        elif j < 5:
                            eng = nc.sync
                        else:
                            eng = nc.scalar
                        eng.dma_start(
                            out=outp[:, o0:o0 + NO * Nt],
                            in_=y_w[:, 0:NO * Nt])

            uT_store = {}
            for j in range(NS):
                mm1(j)
                mm2(j, last=(j == NS - 1))

    nc.compile()
    return nc


def b1t_ap(b1p):
    return b1p[:]


def _route_tokens(xf, Wr, temp):
    """Bit-match the reference's router on CPU jax: logits, top-2, softmax."""
    import jax
    import jax.numpy as jnp

    cpu = jax.devices("cpu")[0]
    with jax.default_device(cpu):
        xj = jnp.asarray(xf)
        logits = (xj @ jnp.asarray(Wr)) / jnp.asarray(temp)
        topw, topi = jax.lax.top_k(logits, TOP_K)
        topw = jax.nn.softmax(topw, axis=-1)
    return np.asarray(topi), np.asarray(topw)


def kernel(**inputs) -> np.ndarray:
    global LAST_RESULTS
    from concourse.bass_utils import run_bass_kernel_spmd

    x = np.asarray(inputs["x"], dtype=np.float32)
    Wr = np.asarray(inputs["Wr"], dtype=np.float32)
    temp = np.asarray(inputs["temp"], dtype=np.float32)
    W1 = np.asarray(inputs["W1"], dtype=np.float32)
    b1 = np.asarray(inputs["b1"], dtype=np.float32)
    W2 = np.asarray(inputs["W2"], dtype=np.float32)
    b2 = np.asarray(inputs["b2"], dtype=np.float32)

    B, S, D = x.shape
    T = B * S
    xf = x.reshape(T, D)

    topi, topw = _route_tokens(xf, Wr, temp)

    # Per-expert token lists and combine weights, sections sorted by
    # descending count (last section smallest -> smallest final tile).
    tok_idx = []
    tok_w = []
    for e in range(NUM_EXPERTS):
        mask = topi == e                       # [T, K]
        sel = mask.any(axis=1)
        idx = np.nonzero(sel)[0]
        w = (topw * mask).sum(axis=1)[idx]
        tok_idx.append(idx)
        tok_w.append(w.astype(np.float32))

    counts = np.array([len(i) for i in tok_idx])
    order = list(np.argsort(-counts, kind="stable"))
    Cs = tuple(_pad(counts[e]) for e in order)
    NS = len(order)

    bf16 = ml_dtypes.bfloat16

    # shared (all cores identical) x sections
    xT_arrs = {}
    for j, e in enumerate(order):
        idx = tok_idx[e]
        xg = np.zeros((Cs[j], D), dtype=np.float32)
        xg[: len(idx)] = xf[idx]
        xT_arrs[f"xT{j}"] = np.ascontiguousarray(xg.T).astype(bf16)

    in_maps = []
    for c in range(N_CORES):
        # w1 slice cols per section: [a_{2c} | g_{2c} | a_{2c+1} | g_{2c+1}]
        blocks = (2 * c, 2 * c + 1)
        w1_cols = []
        for jb in blocks:
            w1_cols.append(np.arange(jb * P, (jb + 1) * P))            # a_jb
            w1_cols.append(np.arange(D_FF + jb * P, D_FF + (jb + 1) * P))
        w1_cols = np.concatenate(w1_cols)
        m = dict(xT_arrs)
        m["w1p"] = np.ascontiguousarray(
            np.concatenate([W1[e][:, w1_cols] for e in order], axis=1)
        ).astype(bf16)
        m["w2p"] = np.ascontiguousarray(
            np.concatenate(
                [W2[e][c * FSL:(c + 1) * FSL, :] for e in order], axis=0)
        ).astype(bf16)
        m["b1p"] = np.ascontiguousarray(
            np.concatenate(
                [b1[e][w1_cols].reshape(4, P).T for e in order], axis=1)
        ).astype(np.float32)
        in_maps.append(m)

    if Cs not in _NC_CACHE:
        _NC_CACHE[Cs] = _build_nc(Cs)
    nc = _NC_CACHE[Cs]

    trace = bool(os.environ.get("MOE_KERNEL_TRACE"))
    kwargs = {}
    if trace:
        kwargs = dict(trace=True, trace_cores=list(range(N_CORES)))
    res = run_bass_kernel_spmd(nc, in_maps, core_ids=list(range(N_CORES)),
                               **kwargs)
    LAST_RESULTS = res

    out_off = []
    w = 0
    for C in Cs:
        out_off.append(w)
        w += NO * C

    out = np.zeros((T, D), dtype=np.float32)
    for j, e in enumerate(order):
        idx = tok_idx[e]
        if len(idx) == 0:
            continue
        C = Cs[j]
        acc = np.zeros((P, NO * C), dtype=np.float32)
        for c in range(N_CORES):
            acc += np.asarray(
                res.results[c]["outp"][:, out_off[j]:out_off[j] + NO * C]
            ).astype(np.float32)
        # per-tile m-major slabs -> [D_MODEL, C]
        y = np.empty((D_MODEL, C), dtype=np.float32)
        for off, Nt in _balanced_tiles(C):
            slab = acc[:, NO * off:NO * (off + Nt)].reshape(P, NO, Nt)
            y[:, off:off + Nt] = slab.transpose(1, 0, 2).reshape(D_MODEL, Nt)
        yt = y[:, : len(idx)].T + b2[e]
        out[idx] += yt * tok_w[e][:, None]

    return out.reshape(B, S, D)


# revision 32
# speedup vs baseline: 1.1626x; 1.0341x over previous
"""Trainium2 Bass kernel for AdaptiveMixtureOfExperts (top-2 SwiGLU MoE).

Strategy (ff-sliced, fully balanced):
  - Host computes the tiny router (x @ Wr, top-2, softmax) with jax-on-CPU ops
    that bit-match the reference, then groups tokens by routed expert.
  - Every core processes EVERY expert's token set, but only a 1/8 slice of
    the FF dimension: core c owns ff neurons [c*256, (c+1)*256) of each
    expert (a-blocks 2c,2c+1 of W1 plus the matching g-blocks, and u-rows
    [c*256,(c+1)*256) of W2).  Section sizes are the true per-expert token
    counts -- identical on every core by construction, so the SPMD graph
    needs NO max-padding (the previous expert-pair layout streamed
    192*(maxbig+maxsmall)=405k columns/core; this streams 48*8192=393k,
    a ~5us saving at the bf16 roofline) and there is no pathological
    ragged tail (tiles are balanced to 342-512 columns).
        hT_slice = W1_sliceT @ xT        (4 psum blocks x 8 k-tiles)
        uT_slice = (a + b1a) * silu(g + b1g)          (2 k-tiles of u)
        y_partial = W2_sliceT @ uT_slice (8 m-blocks x 2 k-tiles)
  - Each core emits a bf16 PARTIAL y (contraction over its 256 u-rows) for
    all token-expert pairs, packed m-major per tile; the host sums the 8
    partials, applies b2 and the top-2 combine weights, and scatter-adds
    into the full [B, S, D] output.

DMA design notes (measured):
  - dma_start issue occupies its HWDGE sequencer ~600ns (128 descriptors)
    regardless of width, so one ring caps at ~206GB/s of 128KB transfers.
  - trn2 has exactly two fast HWDGE rings: qSP (sync) and qACT (scalar).
    The first section's supply is k-split across both (k0-3 on qSP, k4-7 on
    qACT); each ring pays its own ~3us cold ramp and the PE stream-follows
    from ~10.5us.  qACT is clear of issues before the first epilogue ACT.
  - gpsimd/vector dma_start is SWDGE (software Q7 descriptor gen): both
    uses tried in the previous layout REGRESSED (slow Q7 issue + HBM
    contention against the critical qSP head stream).
  - DMA-completion semaphores are 8 round-robin lanes shared by both
    rings; the 9th+ in-flight dma_start blocks on a lane recycling
    (transfer + ~1.2us HBM write receipt).  Fewer, wider early transfers
    beat many narrow ones -- xg moves as one full-section transfer per k.
  - Packed y output (one [128, 8*Nt] DMA per tile, m-major) rides qACT,
    which is otherwise ~50% idle; the global last tile uses per-m DMAs so
    the end-of-kernel exposure is one ~118KB transfer + receipt.
  - The chip clock-states between runs: N=512 matmul spacing is 216ns
    (2.4GHz) on some runs and 259ns (~2.0GHz P0 power state) on others --
    an ~18% exec-time lottery this kernel cannot control.  Compare runs
    only after checking the steady-state matmul delta.

Shapes hardcoded for the problem instance:
  x:[2,2048,1024] f32, Wr:[1024,8], temp:[1], W1:[8,1024,4096], b1:[8,4096],
  W2:[8,2048,1024], b2:[8,1024].  TOP_K=2, 8 experts on 8 cores.
"""

import os

import numpy as np
import ml_dtypes

D_MODEL = 1024
D_FF = 2048
NUM_EXPERTS = 8
TOP_K = 2
P = 128          # partitions
NT = 512         # max token tile (psum bank limit)
N_CORES = 8
FSL = D_FF // N_CORES      # 256 ff neurons per core slice
NO = D_MODEL // P          # 8 output row blocks
K1 = D_MODEL // P          # 8 k-tiles for matmul1
K2 = FSL // P              # 2 k-tiles for matmul2
WSL = 4 * P                # 512 w1 cols per (section, core): a|g|a|g

_NC_CACHE = {}
LAST_RESULTS = None  # test harness introspection


def _balanced_tiles(C):
    """Split C columns into ceil(C/NT) near-equal tiles (all <= NT)."""
    n_t = (C + NT - 1) // NT
    base, rem = divmod(C, n_t)
    sizes = [base + 1] * rem + [base] * (n_t - rem)
    bounds = []
    off = 0
    for sz in sizes:
        bounds.append((off, sz))
        off += sz
    return bounds


def _pad(n):
    # pad only to 4 columns (8-byte bf16 DMA lines); coarser padding streams
    # pure-zero columns through every matmul at ~80ns/column
    return max(P, ((n + 3) // 4) * 4)


def _build_nc(Cs):
    """Per-core Bass graph: NUM_EXPERTS ff-slice FFN sections of Cs[j] tokens.

    Identical on every core; only the dram inputs differ per core.
    """
    import concourse.mybir as mybir
    import concourse.tile as tile
    from concourse import bacc

    f32 = mybir.dt.float32
    bf16 = mybir.dt.bfloat16
    AF = mybir.ActivationFunctionType

    NS = len(Cs)
    Cmax = max(Cs)
    sec_bounds = [_balanced_tiles(C) for C in Cs]
    out_off = []
    w = 0
    for C in Cs:
        out_off.append(w)
        w += NO * C
    W_OUT = w

    nc = bacc.Bacc()
    xT = [
        nc.declare_dram_parameter(f"xT{j}", [D_MODEL, Cs[j]], bf16,
                                  isOutput=False)
        for j in range(NS)
    ]
    w1p = nc.declare_dram_parameter("w1p", [D_MODEL, NS * WSL], bf16,
                                    isOutput=False)
    w2p = nc.declare_dram_parameter("w2p", [NS * FSL, D_MODEL], bf16,
                                    isOutput=False)
    b1p = nc.declare_dram_parameter("b1p", [P, 4 * NS], f32, isOutput=False)
    outp = nc.declare_dram_parameter("outp", [P, W_OUT], bf16, isOutput=True)

    with tile.TileContext(nc) as tc:
        with (
            tc.tile_pool(name="weights", bufs=1) as wpool,
            tc.tile_pool(name="xgs", bufs=3) as xpool,
            tc.tile_pool(name="acts", bufs=2) as upool,
            tc.tile_pool(name="epilogue", bufs=4) as epool,
            tc.tile_pool(name="ps", bufs=8, space="PSUM") as ps_pool,
        ):
            b1_sb = wpool.tile([P, 4 * NS], f32, name="b1_sb", tag="b1")
            w1_sb = [
                wpool.tile([P, NS * WSL], bf16, name=f"w1_sb{k}", tag=f"w1{k}")
                for k in range(K1)
            ]
            w2_sb = [
                wpool.tile([P, D_MODEL], bf16, name=f"w2_sb{s2}",
                           tag=f"w2{s2}")
                for s2 in range(NS * K2)
            ]
            # xg tiles: uniform Cmax width so each k-tag rotates through 3
            # buffers (section j+3's DMA waits for mm1-j to release a slot;
            # with ~20us of compute per section the supply stays 2-3 sections
            # ahead of the PE).  Only Cs[j] columns are ever transferred.
            xg_sb = {}

            def xg_tile(j, k):
                t = xpool.tile([P, Cmax], bf16, name=f"xg{j}_{k}",
                               tag=f"xg{k}", bufs=3)
                xg_sb[(j, k)] = t
                return t

            # ---- input DMAs ----
            def emit_xg(j, eng):
                for k in range(K1):
                    eng.dma_start(
                        out=xg_tile(j, k)[:, 0:Cs[j]],
                        in_=xT[j][k * P:(k + 1) * P, 0:Cs[j]],
                    )

            def emit_w1(j, eng, ks=range(K1)):
                for k in ks:
                    eng.dma_start(
                        out=w1_sb[k][:, j * WSL:(j + 1) * WSL],
                        in_=w1p[k * P:(k + 1) * P, j * WSL:(j + 1) * WSL],
                    )

            def emit_w2(j, eng):
                for kk in range(K2):
                    s2 = j * K2 + kk
                    eng.dma_start(
                        out=w2_sb[s2][:],
                        in_=w2p[s2 * P:(s2 + 1) * P, :],
                    )

            # head: section 0 k-split across both HWDGE rings, xg+w1
            # interleaved per k in PE consumption order; b1p rides qACT
            # early (tiny, needed at the first ACT ~14us).
            Nt00 = sec_bounds[0][0][1]
            for k in range(K1):
                eng = nc.sync if k < K1 // 2 else nc.scalar
                # only tile 0's xg slice rides the critical head window;
                # the rest of section 0 follows right behind.
                eng.dma_start(
                    out=xg_tile(0, k)[:, 0:Nt00],
                    in_=xT[0][k * P:(k + 1) * P, 0:Nt00],
                )
                eng.dma_start(
                    out=w1_sb[k][:, 0:WSL],
                    in_=w1p[k * P:(k + 1) * P, 0:WSL],
                )
                if k == 5:
                    nc.scalar.dma_start(out=b1_sb[:], in_=b1t_ap(b1p))
            for k in range(K1):
                eng = nc.sync if k < K1 // 2 else nc.scalar
                eng.dma_start(
                    out=xg_sb[(0, k)][:, Nt00:Cs[0]],
                    in_=xT[0][k * P:(k + 1) * P, Nt00:Cs[0]],
                )
            # everything else on qSP in demand order
            emit_w2(0, nc.sync)
            for j in range(1, NS):
                emit_w1(j, nc.sync)
                emit_xg(j, nc.sync)
                emit_w2(j, nc.sync)

            # ---- compute ----
            def mm1(j):
                for t, (off, Nt) in enumerate(sec_bounds[j]):
                    uT = upool.tile([P, K2, NT], bf16, name=f"uT{j}_{t}",
                                    tag="uT", bufs=3)
                    uT_store[(j, t)] = uT
                    for p in range(K2):
                        base = j * WSL + 2 * P * p
                        ps_a = ps_pool.tile([P, NT], f32,
                                            name=f"psa{j}_{t}_{p}", tag="ps")
                        ps_g = ps_pool.tile([P, NT], f32,
                                            name=f"psg{j}_{t}_{p}", tag="ps")
                        for k in range(K1):
                            nc.tensor.matmul(
                                ps_a[:, :Nt],
                                w1_sb[k][:, base:base + P],
                                xg_sb[(j, k)][:, off:off + Nt],
                                start=(k == 0), stop=(k == K1 - 1),
                            )
                        for k in range(K1):
                            nc.tensor.matmul(
                                ps_g[:, :Nt],
                                w1_sb[k][:, base + P:base + 2 * P],
                                xg_sb[(j, k)][:, off:off + Nt],
                                start=(k == 0), stop=(k == K1 - 1),
                            )
                        bb = 4 * j + 2 * p
                        a_t = epool.tile([P, NT], bf16, name=f"a{j}_{t}_{p}",
                                         tag="a")
                        # a-side drain on DVE so ACT only carries silu-g:
                        # banks free ~2x faster when the drain backlog is
                        # what bounds the PE.
                        nc.vector.tensor_scalar_add(
                            a_t[:, :Nt], ps_a[:, :Nt], b1_sb[:, bb:bb + 1],
                        )
                        g_t = epool.tile([P, NT], bf16, name=f"g{j}_{t}_{p}",
                                         tag="g")
                        nc.scalar.activation(
                            g_t[:, :Nt], ps_g[:, :Nt], AF.Silu,
                            bias=b1_sb[:, bb + 1:bb + 2],
                        )
                        nc.vector.tensor_mul(
                            uT[:, p, :Nt], a_t[:, :Nt], g_t[:, :Nt])

            def mm2(j, last=False):
                for t, (off, Nt) in enumerate(sec_bounds[j]):
                    uT = uT_store.pop((j, t))
                    # the whole LAST section takes the qSP pair-packed path:
                    # any packed slab on the SWDGE queue near the kernel end
                    # gated the final barrier on SWDGE receipt semaphores
                    # (~2.5us observed), and a per-m path serialized 8 issues
                    is_last = last
                    o0 = out_off[j] + NO * off
                    y_w = None
                    if not is_last:
                        # 2D tile, m-blocks packed contiguously at Nt (not
                        # NT) stride: the output DMA src stays a contiguous
                        # 2D AP (a strided 3D DMA AP costs 3-8us of
                        # descriptor generation).
                        # bufs=3: with 2, tile t's drains wait on tile t-2's
                        # 1MB output DMA receipt (~3-4us); DVE/ACT are
                        # strict-FIFO so the whole drain chain and then the
                        # PE (PSUM banks) stall behind that wait.
                        y_w = epool.tile([P, NO * NT], bf16,
                                         name=f"yw{j}_{t}", tag="yw", bufs=4)
                    for m in range(NO):
                        ps_y = ps_pool.tile([P, NT], f32,
                                            name=f"psy{j}_{t}_{m}", tag="ps")
                        for kk in range(K2):
                            nc.tensor.matmul(
                                ps_y[:, :Nt],
                                w2_sb[j * K2 + kk][:, m * P:(m + 1) * P],
                                uT[:, kk, :Nt],
                                start=(kk == 0), stop=(kk == K2 - 1),
                            )
                        if is_last:
                            # last section: m-pairs packed contiguously at
                            # Nt stride into [P, 2*Nt] tiles, one qSP DMA
                            # per pair (236KB) -- 4 issues/tile, end-of-
                            # kernel exposure is one drain + issue +
                            # transfer + receipt.  Drains alternate DVE/ACT
                            # (different PSUM banks, parallel engines).
                            if m % 2 == 0:
                                y_p = epool.tile([P, 2 * NT], bf16,
                                                 name=f"yp{j}_{t}_{m}",
                                                 tag="yp", bufs=4)
                                nc.vector.tensor_copy(y_p[:, 0:Nt],
                                                      ps_y[:, :Nt])
                            else:
                                nc.scalar.activation(y_p[:, Nt:2 * Nt],
                                                     ps_y[:, :Nt],
                                                     AF.Identity)
                                nc.sync.dma_start(
                                    out=outp[:, o0 + (m - 1) * Nt:
                                             o0 + (m + 1) * Nt],
                                    in_=y_p[:, 0:2 * Nt],
                                )
                        elif m % 2 == 0:
                            # drains alternate DVE / ACT (different PSUM
                            # banks, parallel engines): mm2's drain demand
                            # (8 copies/tile) would otherwise outpace DVE.
                            nc.vector.tensor_copy(
                                y_w[:, m * Nt:(m + 1) * Nt], ps_y[:, :Nt])
                        else:
                            nc.scalar.activation(
                                y_w[:, m * Nt:(m + 1) * Nt], ps_y[:, :Nt],
                                AF.Identity)
                    if not is_last:
                        # one packed m-major DMA per tile.  Ring choice is
                        # phase-dependent: while the input stream owns the 8
                        # HWDGE completion-semaphore lanes (~first 70us), a
                        # qACT issue blocks the strict-FIFO drains ~5us per
                        # section waiting on a lane recycle, so early
                        # sections ride the idle GpSimd SWDGE queue; but the
                        # SWDGE ring drains slowly (its 16MB backlog gated
                        # the final barrier by ~3us), so once inputs are
                        # done, late sections ride qACT (lanes now free).
                        # qSP never carries slabs (in-order ring: +40us of
                        # input stalls when tried).
                        # j>=5 on qACT, not 4: on cores with slower input
                        # streams the lanes are still recycling inputs at
                        # ~120us and a section-4 qACT slab stalled the
                        # drains 8us.  Sections 3-4 ride qSP: by ~90us all
                        # input ISSUES are done, so a slab there delays only
                        # later slabs (early qSP slabs ahead of inputs cost
                        # +40us).  Only sections 0-2 (8.9MB) stay on the
                        # slow (~80GB/s) SWDGE ring so its backlog drains by
                        # ~136us -- with 0-4 on it, the final barrier waited
                        # ~3us on SWDGE receipts after compute ended.
                        if j < 4:
                            eng = nc.gpsimd
                        elif j < 5:
                            eng = nc.sync
                        else:
                            eng = nc.scalar
                        eng.dma_start(
                            out=outp[:, o0:o0 + NO * Nt],
                            in_=y_w[:, 0:NO * Nt])

            uT_store = {}
            for j in range(NS):
                mm1(j)
                mm2(j, last=(j == NS - 1))

    nc.compile()
    return nc


def b1t_ap(b1p):
    return b1p[:]


def _route_tokens(xf, Wr, temp):
    """Bit-match the reference's router on CPU jax: logits, top-2, softmax."""
    import jax
    import jax.numpy as jnp

    cpu = jax.devices("cpu")[0]
    with jax.default_device(cpu):
        xj = jnp.asarray(xf)
        logits = (xj @ jnp.asarray(Wr)) / jnp.asarray(temp)
        topw, topi = jax.lax.top_k(logits, TOP_K)
        topw = jax.nn.softmax(topw, axis=-1)
    return np.asarray(topi), np.asarray(topw)


def kernel(**inputs) -> np.ndarray:
    global LAST_RESULTS
    from concourse.bass_utils import run_bass_kernel_spmd

    x = np.asarray(inputs["x"], dtype=np.float32)
    Wr = np.asarray(inputs["Wr"], dtype=np.float32)
    temp = np.asarray(inputs["temp"], dtype=np.float32)
    W1 = np.asarray(inputs["W1"], dtype=np.float32)
    b1 = np.asarray(inputs["b1"], dtype=np.float32)
    W2 = np.asarray(inputs["W2"], dtype=np.float32)
    b2 = np.asarray(inputs["b2"], dtype=np.float32)

    B, S, D = x.shape
    T = B * S
    xf = x.reshape(T, D)

    topi, topw = _route_tokens(xf, Wr, temp)

    # Per-expert token lists and combine weights, sections sorted by
    # descending count (last section smallest -> smallest final tile).
    tok_idx = []
    tok_w = []
    for e in range(NUM_EXPERTS):
        mask = topi == e                       # [T, K]
        sel = mask.any(axis=1)
        idx = np.nonzero(sel)[0]
        w = (topw * mask).sum(axis=1)[idx]
        tok_idx.append(idx)
        tok_w.append(w.astype(np.float32))

    counts = np.array([len(i) for i in tok_idx])
    order = list(np.argsort(-counts, kind="stable"))
    Cs = tuple(_pad(counts[e]) for e in order)
    NS = len(order)

    bf16 = ml_dtypes.bfloat16

    # shared (all cores identical) x sections
    xT_arrs = {}
    for j, e in enumerate(order):
        idx = tok_idx[e]
        xg = np.zeros((Cs[j], D), dtype=np.float32)
        xg[: len(idx)] = xf[idx]
        xT_arrs[f"xT{j}"] = np.ascontiguousarray(xg.T).astype(bf16)

    in_maps = []
    for c in range(N_CORES):
        # w1 slice cols per section: [a_{2c} | g_{2c} | a_{2c+1} | g_{2c+1}]
        blocks = (2 * c, 2 * c + 1)
        w1_cols = []
        for jb in blocks:
            w1_cols.append(np.arange(jb * P, (jb + 1) * P))            # a_jb
            w1_cols.append(np.arange(D_FF + jb * P, D_FF + (jb + 1) * P))
        w1_cols = np.concatenate(w1_cols)
        m = dict(xT_arrs)
        m["w1p"] = np.ascontiguousarray(
            np.concatenate([W1[e][:, w1_cols] for e in order], axis=1)
        ).astype(bf16)
        m["w2p"] = np.ascontiguousarray(
            np.concatenate(
                [W2[e][c * FSL:(c + 1) * FSL, :] for e in order], axis=0)
        ).astype(bf16)
        m["b1p"] = np.ascontiguousarray(
            np.concatenate(
                [b1[e][w1_cols].reshape(4, P).T for e in order], axis=1)
        ).astype(np.float32)
        in_maps.append(m)

    if Cs not in _NC_CACHE:
        _NC_CACHE[Cs] = _build_nc(Cs)
    nc = _NC_CACHE[Cs]

    trace = bool(os.environ.get("MOE_KERNEL_TRACE"))
    kwargs = {}
    if trace:
        kwargs = dict(trace=True, trace_cores=list(range(N_CORES)))
    res = run_bass_kernel_spmd(nc, in_maps, core_ids=list(range(N_CORES)),
                               **kwargs)
    LAST_RESULTS = res

    out_off = []
    w = 0
    for C in Cs:
        out_off.append(w)
        w += NO * C

    out = np.zeros((T, D), dtype=np.float32)
    for j, e in enumerate(order):
        idx = tok_idx[e]
        if len(idx) == 0:
            continue
        C = Cs[j]
        acc = np.zeros((P, NO * C), dtype=np.float32)
        for c in range(N_CORES):
            acc += np.asarray(
                res.results[c]["outp"][:, out_off[j]:out_off[j] + NO * C]
            ).astype(np.float32)
        # per-tile m-major slabs -> [D_MODEL, C]
        y = np.empty((D_MODEL, C), dtype=np.float32)
        for off, Nt in _balanced_tiles(C):
            slab = acc[:, NO * off:NO * (off + Nt)].reshape(P, NO, Nt)
            y[:, off:off + Nt] = slab.transpose(1, 0, 2).reshape(D_MODEL, Nt)
        yt = y[:, : len(idx)].T + b2[e]
        out[idx] += yt * tok_w[e][:, None]

    return out.reshape(B, S, D)


# revision 34
# speedup vs baseline: 1.1984x; 1.0308x over previous
"""Trainium2 Bass kernel for AdaptiveMixtureOfExperts (top-2 SwiGLU MoE).

Strategy (ff-sliced, fully balanced):
  - Host computes the tiny router (x @ Wr, top-2, softmax) with jax-on-CPU ops
    that bit-match the reference, then groups tokens by routed expert.
  - Every core processes EVERY expert's token set, but only a 1/8 slice of
    the FF dimension: core c owns ff neurons [c*256, (c+1)*256) of each
    expert (a-blocks 2c,2c+1 of W1 plus the matching g-blocks, and u-rows
    [c*256,(c+1)*256) of W2).  Section sizes are the true per-expert token
    counts -- identical on every core by construction, so the SPMD graph
    needs NO max-padding (the previous expert-pair layout streamed
    192*(maxbig+maxsmall)=405k columns/core; this streams 48*8192=393k,
    a ~5us saving at the bf16 roofline) and there is no pathological
    ragged tail (tiles are balanced to 342-512 columns).
        hT_slice = W1_sliceT @ xT        (4 psum blocks x 8 k-tiles)
        uT_slice = (a + b1a) * silu(g + b1g)          (2 k-tiles of u)
        y_partial = W2_sliceT @ uT_slice (8 m-blocks x 2 k-tiles)
  - Each core emits a bf16 PARTIAL y (contraction over its 256 u-rows) for
    all token-expert pairs, packed m-major per tile; the host sums the 8
    partials, applies b2 and the top-2 combine weights, and scatter-adds
    into the full [B, S, D] output.

DMA design notes (measured):
  - dma_start issue occupies its HWDGE sequencer ~600ns (128 descriptors)
    regardless of width, so one ring caps at ~206GB/s of 128KB transfers.
  - trn2 has exactly two fast HWDGE rings: qSP (sync) and qACT (scalar).
    The first section's supply is k-split across both (k0-3 on qSP, k4-7 on
    qACT); each ring pays its own ~3us cold ramp and the PE stream-follows
    from ~10.5us.  qACT is clear of issues before the first epilogue ACT.
  - gpsimd/vector dma_start is SWDGE (software Q7 descriptor gen): both
    uses tried in the previous layout REGRESSED (slow Q7 issue + HBM
    contention against the critical qSP head stream).
  - DMA-completion semaphores are 8 round-robin lanes shared by both
    rings; the 9th+ in-flight dma_start blocks on a lane recycling
    (transfer + ~1.2us HBM write receipt).  Fewer, wider early transfers
    beat many narrow ones -- xg moves as one full-section transfer per k.
  - Packed y output (one [128, 8*Nt] DMA per tile, m-major) rides qACT,
    which is otherwise ~50% idle; the global last tile uses per-m DMAs so
    the end-of-kernel exposure is one ~118KB transfer + receipt.
  - The chip clock-states between runs: N=512 matmul spacing is 216ns
    (2.4GHz) on some runs and 259ns (~2.0GHz P0 power state) on others --
    an ~18% exec-time lottery this kernel cannot control.  Compare runs
    only after checking the steady-state matmul delta.

Shapes hardcoded for the problem instance:
  x:[2,2048,1024] f32, Wr:[1024,8], temp:[1], W1:[8,1024,4096], b1:[8,4096],
  W2:[8,2048,1024], b2:[8,1024].  TOP_K=2, 8 experts on 8 cores.
"""

import os

import numpy as np
import ml_dtypes

D_MODEL = 1024
D_FF = 2048
NUM_EXPERTS = 8
TOP_K = 2
P = 128          # partitions
NT = 512         # max token tile (psum bank limit)
N_CORES = 8
FSL = D_FF // N_CORES      # 256 ff neurons per core slice
NO = D_MODEL // P          # 8 output row blocks
K1 = D_MODEL // P          # 8 k-tiles for matmul1
K2 = FSL // P              # 2 k-tiles for matmul2
WSL = 4 * P                # 512 w1 cols per (section, core): a|g|a|g

_NC_CACHE = {}
LAST_RESULTS = None  # test harness introspection


def _balanced_tiles(C):
    """Split C columns into ceil(C/NT) near-equal tiles (all <= NT)."""
    n_t = (C + NT - 1) // NT
    base, rem = divmod(C, n_t)
    sizes = [base + 1] * rem + [base] * (n_t - rem)
    bounds = []
    off = 0
    for sz in sizes:
        bounds.append((off, sz))
        off += sz
    return bounds


def _pad(n):
    # pad only to 4 columns (8-byte bf16 DMA lines); coarser padding streams
    # pure-zero columns through every matmul at ~80ns/column
    return max(P, ((n + 3) // 4) * 4)


def _build_nc(Cs):
    """Per-core Bass graph: NUM_EXPERTS ff-slice FFN sections of Cs[j] tokens.

    Identical on every core; only the dram inputs differ per core.
    """
    import concourse.mybir as mybir
    import concourse.tile as tile
    from concourse import bacc

    f32 = mybir.dt.float32
    bf16 = mybir.dt.bfloat16
    AF = mybir.ActivationFunctionType

    NS = len(Cs)
    Cmax = max(Cs)
    sec_bounds = [_balanced_tiles(C) for C in Cs]
    out_off = []
    w = 0
    for C in Cs:
        out_off.append(w)
        w += NO * C
    W_OUT = w

    nc = bacc.Bacc()
    xT = [
        nc.declare_dram_parameter(f"xT{j}", [D_MODEL, Cs[j]], bf16,
                                  isOutput=False)
        for j in range(NS)
    ]
    w1p = nc.declare_dram_parameter("w1p", [D_MODEL, NS * WSL], bf16,
                                    isOutput=False)
    w2p = nc.declare_dram_parameter("w2p", [NS * FSL, D_MODEL], bf16,
                                    isOutput=False)
    b1p = nc.declare_dram_parameter("b1p", [P, 4 * NS], f32, isOutput=False)
    outp = nc.declare_dram_parameter("outp", [P, W_OUT], bf16, isOutput=True)

    with tile.TileContext(nc) as tc:
        with (
            tc.tile_pool(name="weights", bufs=1) as wpool,
            tc.tile_pool(name="xgs", bufs=3) as xpool,
            tc.tile_pool(name="acts", bufs=2) as upool,
            tc.tile_pool(name="epilogue", bufs=4) as epool,
            tc.tile_pool(name="ps", bufs=8, space="PSUM") as ps_pool,
        ):
            b1_sb = wpool.tile([P, 4 * NS], f32, name="b1_sb", tag="b1")
            w1_sb = [
                wpool.tile([P, NS * WSL], bf16, name=f"w1_sb{k}", tag=f"w1{k}")
                for k in range(K1)
            ]
            w2_sb = [
                wpool.tile([P, D_MODEL], bf16, name=f"w2_sb{s2}",
                           tag=f"w2{s2}")
                for s2 in range(NS * K2)
            ]
            # xg tiles: uniform Cmax width so each k-tag rotates through 3
            # buffers (section j+3's DMA waits for mm1-j to release a slot;
            # with ~20us of compute per section the supply stays 2-3 sections
            # ahead of the PE).  Only Cs[j] columns are ever transferred.
            xg_sb = {}

            def xg_tile(j, k):
                t = xpool.tile([P, Cmax], bf16, name=f"xg{j}_{k}",
                               tag=f"xg{k}", bufs=3)
                xg_sb[(j, k)] = t
                return t

            # ---- input DMAs ----
            def emit_xg(j, eng):
                for k in range(K1):
                    eng.dma_start(
                        out=xg_tile(j, k)[:, 0:Cs[j]],
                        in_=xT[j][k * P:(k + 1) * P, 0:Cs[j]],
                    )

            def emit_w1(j, eng, ks=range(K1)):
                for k in ks:
                    eng.dma_start(
                        out=w1_sb[k][:, j * WSL:(j + 1) * WSL],
                        in_=w1p[k * P:(k + 1) * P, j * WSL:(j + 1) * WSL],
                    )

            def emit_w2(j, eng):
                for kk in range(K2):
                    s2 = j * K2 + kk
                    eng.dma_start(
                        out=w2_sb[s2][:],
                        in_=w2p[s2 * P:(s2 + 1) * P, :],
                    )

            # head: section 0 k-split across both HWDGE rings, xg+w1
            # interleaved per k in PE consumption order; b1p rides qACT
            # early (tiny, needed at the first ACT ~14us).
            Nt00 = sec_bounds[0][0][1]
            for k in range(K1):
                eng = nc.sync if k < K1 // 2 else nc.scalar
                # only tile 0's xg slice rides the critical head window;
                # the rest of section 0 follows right behind.
                eng.dma_start(
                    out=xg_tile(0, k)[:, 0:Nt00],
                    in_=xT[0][k * P:(k + 1) * P, 0:Nt00],
                )
                eng.dma_start(
                    out=w1_sb[k][:, 0:WSL],
                    in_=w1p[k * P:(k + 1) * P, 0:WSL],
                )
                if k == 5:
                    nc.scalar.dma_start(out=b1_sb[:], in_=b1t_ap(b1p))
            for k in range(K1):
                eng = nc.sync if k < K1 // 2 else nc.scalar
                eng.dma_start(
                    out=xg_sb[(0, k)][:, Nt00:Cs[0]],
                    in_=xT[0][k * P:(k + 1) * P, Nt00:Cs[0]],
                )
            # everything else on qSP in demand order
            emit_w2(0, nc.sync)
            for j in range(1, NS):
                emit_w1(j, nc.sync)
                emit_xg(j, nc.sync)
                emit_w2(j, nc.sync)

            # ---- compute ----
            def mm1(j):
                for t, (off, Nt) in enumerate(sec_bounds[j]):
                    uT = upool.tile([P, K2, NT], bf16, name=f"uT{j}_{t}",
                                    tag="uT", bufs=3)
                    uT_store[(j, t)] = uT
                    for p in range(K2):
                        base = j * WSL + 2 * P * p
                        ps_a = ps_pool.tile([P, NT], f32,
                                            name=f"psa{j}_{t}_{p}", tag="ps")
                        ps_g = ps_pool.tile([P, NT], f32,
                                            name=f"psg{j}_{t}_{p}", tag="ps")
                        for k in range(K1):
                            nc.tensor.matmul(
                                ps_a[:, :Nt],
                                w1_sb[k][:, base:base + P],
                                xg_sb[(j, k)][:, off:off + Nt],
                                start=(k == 0), stop=(k == K1 - 1),
                            )
                        for k in range(K1):
                            nc.tensor.matmul(
                                ps_g[:, :Nt],
                                w1_sb[k][:, base + P:base + 2 * P],
                                xg_sb[(j, k)][:, off:off + Nt],
                                start=(k == 0), stop=(k == K1 - 1),
                            )
                        bb = 4 * j + 2 * p
                        a_t = epool.tile([P, NT], bf16, name=f"a{j}_{t}_{p}",
                                         tag="a")
                        # a-side drain on DVE so ACT only carries silu-g:
                        # banks free ~2x faster when the drain backlog is
                        # what bounds the PE.
                        nc.vector.tensor_scalar_add(
                            a_t[:, :Nt], ps_a[:, :Nt], b1_sb[:, bb:bb + 1],
                        )
                        g_t = epool.tile([P, NT], bf16, name=f"g{j}_{t}_{p}",
                                         tag="g")
                        nc.scalar.activation(
                            g_t[:, :Nt], ps_g[:, :Nt], AF.Silu,
                            bias=b1_sb[:, bb + 1:bb + 2],
                        )
                        nc.vector.tensor_mul(
                            uT[:, p, :Nt], a_t[:, :Nt], g_t[:, :Nt])

            def mm2(j, last=False):
                for t, (off, Nt) in enumerate(sec_bounds[j]):
                    uT = uT_store.pop((j, t))
                    # the whole LAST section takes the qSP pair-packed path:
                    # any packed slab on the SWDGE queue near the kernel end
                    # gated the final barrier on SWDGE receipt semaphores
                    # (~2.5us observed), and a per-m path serialized 8 issues
                    is_last = last
                    o0 = out_off[j] + NO * off
                    y_w = None
                    if not is_last:
                        # 2D tile, m-blocks packed contiguously at Nt (not
                        # NT) stride: the output DMA src stays a contiguous
                        # 2D AP (a strided 3D DMA AP costs 3-8us of
                        # descriptor generation).
                        # bufs=3: with 2, tile t's drains wait on tile t-2's
                        # 1MB output DMA receipt (~3-4us); DVE/ACT are
                        # strict-FIFO so the whole drain chain and then the
                        # PE (PSUM banks) stall behind that wait.
                        y_w = epool.tile([P, NO * NT], bf16,
                                         name=f"yw{j}_{t}", tag="yw", bufs=4)
                    for m in range(NO):
                        ps_y = ps_pool.tile([P, NT], f32,
                                            name=f"psy{j}_{t}_{m}", tag="ps")
                        for kk in range(K2):
                            nc.tensor.matmul(
                                ps_y[:, :Nt],
                                w2_sb[j * K2 + kk][:, m * P:(m + 1) * P],
                                uT[:, kk, :Nt],
                                start=(kk == 0), stop=(kk == K2 - 1),
                            )
                        if is_last:
                            # last section: m-pairs packed contiguously at
                            # Nt stride into [P, 2*Nt] tiles, one qSP DMA
                            # per pair (236KB) -- 4 issues/tile, end-of-
                            # kernel exposure is one drain + issue +
                            # transfer + receipt.  Drains alternate DVE/ACT
                            # (different PSUM banks, parallel engines).
                            if m % 2 == 0:
                                y_p = epool.tile([P, 2 * NT], bf16,
                                                 name=f"yp{j}_{t}_{m}",
                                                 tag="yp", bufs=4)
                                nc.vector.tensor_copy(y_p[:, 0:Nt],
                                                      ps_y[:, :Nt])
                            else:
                                nc.scalar.activation(y_p[:, Nt:2 * Nt],
                                                     ps_y[:, :Nt],
                                                     AF.Identity)
                                nc.sync.dma_start(
                                    out=outp[:, o0 + (m - 1) * Nt:
                                             o0 + (m + 1) * Nt],
                                    in_=y_p[:, 0:2 * Nt],
                                )
                        elif m % 2 == 0:
                            # drains alternate DVE / ACT (different PSUM
                            # banks, parallel engines): mm2's drain demand
                            # (8 copies/tile) would otherwise outpace DVE.
                            nc.vector.tensor_copy(
                                y_w[:, m * Nt:(m + 1) * Nt], ps_y[:, :Nt])
                        else:
                            nc.scalar.activation(
                                y_w[:, m * Nt:(m + 1) * Nt], ps_y[:, :Nt],
                                AF.Identity)
                    if not is_last:
                        # one packed m-major DMA per tile.  Ring choice is
                        # phase-dependent: while the input stream owns the 8
                        # HWDGE completion-semaphore lanes (~first 70us), a
                        # qACT issue blocks the strict-FIFO drains ~5us per
                        # section waiting on a lane recycle, so early
                        # sections ride the idle GpSimd SWDGE queue; but the
                        # SWDGE ring drains slowly (its 16MB backlog gated
                        # the final barrier by ~3us), so once inputs are
                        # done, late sections ride qACT (lanes now free).
                        # qSP never carries slabs (in-order ring: +40us of
                        # input stalls when tried).
                        # j>=5 on qACT, not 4: on cores with slower input
                        # streams the lanes are still recycling inputs at
                        # ~120us and a section-4 qACT slab stalled the
                        # drains 8us.  Sections 3-4 ride qSP: by ~90us all
                        # input ISSUES are done, so a slab there delays only
                        # later slabs (early qSP slabs ahead of inputs cost
                        # +40us).  Only sections 0-2 (8.9MB) stay on the
                        # slow (~80GB/s) SWDGE ring so its backlog drains by
                        # ~136us -- with 0-4 on it, the final barrier waited
                        # ~3us on SWDGE receipts after compute ended.
                        if j < 4:
                            eng = nc.gpsimd
                        elif j < 5:
                            eng = nc.sync
                        else:
                            eng = nc.scalar
                        eng.dma_start(
                            out=outp[:, o0:o0 + NO * Nt],
                            in_=y_w[:, 0:NO * Nt])

            uT_store = {}
            for j in range(NS):
                mm1(j)
                mm2(j, last=(j == NS - 1))

    nc.compile()
    return nc


def b1t_ap(b1p):
    return b1p[:]


def _route_tokens(xf, Wr, temp):
    """Bit-match the reference's router on CPU jax: logits, top-2, softmax."""
    import jax
    import jax.numpy as jnp

    cpu = jax.devices("cpu")[0]
    with jax.default_device(cpu):
        xj = jnp.asarray(xf)
        logits = (xj @ jnp.asarray(Wr)) / jnp.asarray(temp)
        topw, topi = jax.lax.top_k(logits, TOP_K)
        topw = jax.nn.softmax(topw, axis=-1)
    return np.asarray(topi), np.asarray(topw)


def kernel(**inputs) -> np.ndarray:
    global LAST_RESULTS
    from concourse.bass_utils import run_bass_kernel_spmd

    x = np.asarray(inputs["x"], dtype=np.float32)
    Wr = np.asarray(inputs["Wr"], dtype=np.float32)
    temp = np.asarray(inputs["temp"], dtype=np.float32)
    W1 = np.asarray(inputs["W1"], dtype=np.float32)
    b1 = np.asarray(inputs["b1"], dtype=np.float32)
    W2 = np.asarray(inputs["W2"], dtype=np.float32)
    b2 = np.asarray(inputs["b2"], dtype=np.float32)

    B, S, D = x.shape
    T = B * S
    xf = x.reshape(T, D)

    topi, topw = _route_tokens(xf, Wr, temp)

    # Per-expert token lists and combine weights, sections sorted by
    # descending count (last section smallest -> smallest final tile).
    tok_idx = []
    tok_w = []
    for e in range(NUM_EXPERTS):
        mask = topi == e                       # [T, K]
        sel = mask.any(axis=1)
        idx = np.nonzero(sel)[0]
        w = (topw * mask).sum(axis=1)[idx]
        tok_idx.append(idx)
        tok_w.append(w.astype(np.float32))

    counts = np.array([len(i) for i in tok_idx])
    order = list(np.argsort(-counts, kind="stable"))
    Cs = tuple(_pad(counts[e]) for e in order)
    NS = len(order)

    bf16 = ml_dtypes.bfloat16

    # shared (all cores identical) x sections
    xT_arrs = {}
    for j, e in enumerate(order):
        idx = tok_idx[e]
        xg = np.zeros((Cs[j], D), dtype=np.float32)
        xg[: len(idx)] = xf[idx]
        xT_arrs[f"xT{j}"] = np.ascontiguousarray(xg.T).astype(bf16)

    in_maps = []
    for c in range(N_CORES):
        # w1 slice cols per section: [a_{2c} | g_{2c} | a_{2c+1} | g_{2c+1}]
        blocks = (2 * c, 2 * c + 1)
        w1_cols = []
        for jb in blocks:
            w1_cols.append(np.arange(jb * P, (jb + 1) * P))            # a_jb
            w1_cols.append(np.arange(D_FF + jb * P, D_FF + (jb + 1) * P))
        w1_cols = np.concatenate(w1_cols)
        m = dict(xT_arrs)
        m["w1p"] = np.ascontiguousarray(
            np.concatenate([W1[e][:, w1_cols] for e in order], axis=1)
        ).astype(bf16)
        m["w2p"] = np.ascontiguousarray(
            np.concatenate(
                [W2[e][c * FSL:(c + 1) * FSL, :] for e in order], axis=0)
        ).astype(bf16)
        m["b1p"] = np.ascontiguousarray(
            np.concatenate(
                [b1[e][w1_cols].reshape(4, P).T for e in order], axis=1)
        ).astype(np.float32)
        in_maps.append(m)

    if Cs not in _NC_CACHE:
        _NC_CACHE[Cs] = _build_nc(Cs)
    nc = _NC_CACHE[Cs]

    trace = bool(os.environ.get("MOE_KERNEL_TRACE"))
    kwargs = {}
    if trace:
        kwargs = dict(trace=True, trace_cores=list(range(N_CORES)))
    res = run_bass_kernel_spmd(nc, in_maps, core_ids=list(range(N_CORES)),
                               **kwargs)
    LAST_RESULTS = res

    out_off = []
    w = 0
    for C in Cs:
        out_off.append(w)
        w += NO * C

    out = np.zeros((T, D), dtype=np.float32)
    for j, e in enumerate(order):
        idx = tok_idx[e]
        if len(idx) == 0:
            continue
        C = Cs[j]
        acc = np.zeros((P, NO * C), dtype=np.float32)
        for c in range(N_CORES):
            acc += np.asarray(
                res.results[c]["outp"][:, out_off[j]:out_off[j] + NO * C]
            ).astype(np.float32)
        # per-tile m-major slabs -> [D_MODEL, C]
        y = np.empty((D_MODEL, C), dtype=np.float32)
        for off, Nt in _balanced_tiles(C):
            slab = acc[:, NO * off:NO * (off + Nt)].reshape(P, NO, Nt)
            y[:, off:off + Nt] = slab.transpose(1, 0, 2).reshape(D_MODEL, Nt)
        yt = y[:, : len(idx)].T + b2[e]
        out[idx] += yt * tok_w[e][:, None]

    return out.reshape(B, S, D)
